# revision 7
# baseline (speedup 1.0000x reference)
"""Trainium2 Bass kernel for nn_AdaptiveMask: out = x * ring_mask(current_val).

x: [32, 8, 256, 256] f32.  mask: [256, 256] computed from the scalar
current_val (concentric-ring ramp, values in [0, 1]).

Strategy (memory-bound, pure elementwise):
  - Shard x along batch dim: 4 batches per core across 8 cores (data parallel).
  - Host precomputes the [256, 256] mask from current_val, then lays it out as
    a [128, 4096] "mega mask" that matches the SBUF tile layout of a contiguous
    2 MiB chunk of x, so the device does a plain tensor_tensor multiply with no
    broadcast logic.
  - Per core: 4 tiles of [128, 4096] f32 (2 MiB each, contiguous in HBM).
    Loads on nc.sync (HWDGE ring 0), multiply on DVE in-place, stores on
    nc.scalar (HWDGE ring 1). Tile framework handles pipelining (bufs=4).

Layout math: per-core shard [4, 8, 256, 256] viewed as [512, 4096] row-major.
Element (R, j) of that view is image row r = (R % 16) * 16 + j // 256 and
col c = j % 256 (every R spans 16 consecutive image rows; R % 16 is the
16-row group within one 256-row image). A [128, 4096] tile starting at
R = 128 t keeps the same mapping for every t because 128 ≡ 0 (mod 16)...
in fact each tile holds 8 complete [256, 256] images worth of rows, so one
mega-mask M[p, j] = mask[(p % 16) * 16 + j // 256, j % 256] serves all tiles.
"""

import sys

import numpy as np

for _p in ("/opt/trn_rl_repo",):
    if _p not in sys.path:
        sys.path.append(_p)

from concourse import bacc, bass, tile
from concourse.bass import mybir
from concourse.bass_utils import run_bass_kernel_spmd

N_CORES = 8
B, H, N = 32, 8, 256
MAX_SIZE = 256
RAMP_SIZE = 32

ROWS = (B // N_CORES) * H * N * N // 4096  # 512 rows of 4096 f32 per core
TILE_F = 4096
N_TILES = ROWS // 128  # 4

_cache = {}


def _build_program():
    nc = bacc.Bacc(None, target_bir_lowering=False)
    x_in = nc.dram_tensor("x_in", [ROWS, TILE_F], mybir.dt.float32, kind="ExternalInput")
    m_in = nc.dram_tensor("m_in", [16, TILE_F], mybir.dt.float32, kind="ExternalInput")
    out = nc.dram_tensor("out", [ROWS, TILE_F], mybir.dt.float32, kind="ExternalOutput")

    with tile.TileContext(nc) as tc:
        with (
            tc.tile_pool(name="maskp", bufs=1) as mp,
            tc.tile_pool(name="data", bufs=4) as dp,
        ):
            # Load the unique 256 KiB mask into partitions 0:16, then
            # log-double it across all 128 partitions with SBUF->SBUF DMAs.
            # The whole mask chain rides the scalar (ACT) HWDGE ring so its
            # sem-waits never stall the data loads on the sync (SP) ring;
            # stores share the scalar ring but only start much later.
            mt = mp.tile([128, TILE_F], mybir.dt.float32)
            nc.scalar.dma_start(mt[0:16, :], m_in[:])
            nc.scalar.dma_start(mt[16:32, :], mt[0:16, :])
            nc.scalar.dma_start(mt[32:64, :], mt[0:32, :])
            nc.scalar.dma_start(mt[64:128, :], mt[0:64, :])
            for t in range(N_TILES):
                d = dp.tile([128, TILE_F], mybir.dt.float32)
                nc.sync.dma_start(d[:], x_in[t * 128 : (t + 1) * 128, :])
                nc.vector.tensor_mul(d[:], d[:], mt[:])
                nc.scalar.dma_start(out[t * 128 : (t + 1) * 128, :], d[:])
    nc.finalize()
    return nc


def _get_program():
    if "nc" not in _cache:
        _cache["nc"] = _build_program()
    return _cache["nc"]


def _compute_mask(cv: float) -> np.ndarray:
    """Replicates reference's mask math in numpy f32: [N, N]."""
    template = np.linspace(1.0 - MAX_SIZE, 0.0, MAX_SIZE, dtype=np.float32)
    one_d = np.clip(
        (template + np.float32(cv) * MAX_SIZE) / np.float32(RAMP_SIZE) + np.float32(1.0),
        np.float32(0.0),
        np.float32(1.0),
    ).astype(np.float32)
    one_d = one_d[-(N // 2):]  # [128]
    idx = np.arange(N)
    ring = np.minimum(
        np.minimum(idx[:, None], idx[None, :]),
        np.minimum(N - 1 - idx[:, None], N - 1 - idx[None, :]),
    )  # values in [0, 127] for N=256 — always < N//2, no center special case
    return one_d[ring]


def _run(x, current_val, **spmd_kwargs):
    x = np.ascontiguousarray(np.asarray(x), dtype=np.float32)
    cv = float(np.asarray(current_val).reshape(-1)[0])
    assert x.shape == (B, H, N, N), x.shape

    mask = _compute_mask(cv)  # [256, 256]
    # unique mask in tile layout: row p%16 of every SBUF partition group
    m4 = np.ascontiguousarray(mask.reshape(16, TILE_F))

    per_core = B // N_CORES
    in_maps = [
        {
            "x_in": x[c * per_core : (c + 1) * per_core].reshape(ROWS, TILE_F),
            "m_in": m4,
        }
        for c in range(N_CORES)
    ]

    nc = _get_program()
    res = run_bass_kernel_spmd(nc, in_maps, list(range(N_CORES)), **spmd_kwargs)
    out = np.concatenate(
        [r["out"].reshape(per_core, H, N, N) for r in res.results], axis=0
    )
    return out, res


def kernel(x, current_val):
    return _run(x, current_val)[0]


if __name__ == "__main__":
    xs = np.random.randn(B, H, N, N).astype(np.float32)
    cv = np.array([0.1], dtype=np.float32)
    o = kernel(x=xs, current_val=cv)
    expected = xs * _compute_mask(0.1)
    print("self-check max abs diff:", np.abs(o - expected).max())


# revision 10
# speedup vs baseline: 1.2290x; 1.2290x over previous
"""Trainium2 Bass kernel for nn_AdaptiveMask: out = x * ring_mask(current_val).

x: [32, 8, 256, 256] f32.  mask: [256, 256] computed from the scalar
current_val (concentric-ring ramp, values in [0, 1]).

Strategy (memory-bound, pure elementwise):
  - Shard x along batch dim: 4 batches per core across 8 cores (data parallel).
  - Host precomputes the [256, 256] mask from current_val, then lays it out as
    a [128, 4096] "mega mask" that matches the SBUF tile layout of a contiguous
    2 MiB chunk of x, so the device does a plain tensor_tensor multiply with no
    broadcast logic.
  - Per core: 4 tiles of [128, 4096] f32 (2 MiB each, contiguous in HBM).
    Loads on nc.sync (HWDGE ring 0), multiply on DVE in-place, stores on
    nc.scalar (HWDGE ring 1). Tile framework handles pipelining (bufs=4).

Layout math: per-core shard [4, 8, 256, 256] viewed as [512, 4096] row-major.
Element (R, j) of that view is image row r = (R % 16) * 16 + j // 256 and
col c = j % 256 (every R spans 16 consecutive image rows; R % 16 is the
16-row group within one 256-row image). A [128, 4096] tile starting at
R = 128 t keeps the same mapping for every t because 128 ≡ 0 (mod 16)...
in fact each tile holds 8 complete [256, 256] images worth of rows, so one
mega-mask M[p, j] = mask[(p % 16) * 16 + j // 256, j % 256] serves all tiles.
"""

import sys

import numpy as np

for _p in ("/opt/trn_rl_repo",):
    if _p not in sys.path:
        sys.path.append(_p)

from concourse import bacc, bass, tile
from concourse.bass import mybir
from concourse.bass_utils import run_bass_kernel_spmd

N_CORES = 8
B, H, N = 32, 8, 256
MAX_SIZE = 256
RAMP_SIZE = 32

ROWS = (B // N_CORES) * H * N * N // 4096  # 512 rows of 4096 f32 per core
TILE_F = 4096
N_TILES = ROWS // 128  # 4

_cache = {}


def _build_program():
    nc = bacc.Bacc(None, target_bir_lowering=False)
    x_in = nc.dram_tensor("x_in", [ROWS, TILE_F], mybir.dt.float32, kind="ExternalInput")
    m_in = nc.dram_tensor("m_in", [16, TILE_F], mybir.dt.float32, kind="ExternalInput")
    w_in = nc.dram_tensor("w_in", [16, 128], mybir.dt.float32, kind="ExternalInput")
    out = nc.dram_tensor("out", [ROWS, TILE_F], mybir.dt.float32, kind="ExternalOutput")

    with tile.TileContext(nc) as tc:
        with (
            tc.tile_pool(name="maskp", bufs=1) as mp,
            tc.tile_pool(name="data", bufs=4) as dp,
            tc.tile_pool(name="psum", bufs=1, space="PSUM") as pp,
        ):
            # Load the unique 256 KiB mask [16, 4096] plus an 8 KiB 0/1
            # selector [16, 128] (W[k, p] = p % 16 == k), FIRST on the sync
            # ring so they complete before the big data loads hog the SDMA
            # engines. PE broadcasts the mask across all 128 partitions:
            # psum[p, n] = sum_k W[k, p] * m_small[k, n] = mask[p % 16, n]
            # (exact: exactly one nonzero term). One DVE copy lands it in
            # SBUF. This avoids reading a replicated 2 MiB mega-mask from
            # HBM - only 264 KiB of mask traffic per core.
            wt = mp.tile([16, 128], mybir.dt.float32, tag="wsel")
            ms = mp.tile([16, TILE_F], mybir.dt.float32, tag="msmall")
            mt = mp.tile([128, TILE_F], mybir.dt.float32, tag="mask")
            mpsum = pp.tile([128, TILE_F], mybir.dt.float32)
            nc.sync.dma_start(wt[:], w_in[:])
            nc.sync.dma_start(ms[:], m_in[:])
            for k in range(TILE_F // 512):
                nc.tensor.matmul(
                    mpsum[:, k * 512 : (k + 1) * 512],
                    wt[:],
                    ms[:, k * 512 : (k + 1) * 512],
                    start=True,
                    stop=True,
                )
            nc.vector.tensor_copy(mt[:], mpsum[:])
            for t in range(N_TILES):
                d = dp.tile([128, TILE_F], mybir.dt.float32)
                nc.sync.dma_start(d[:], x_in[t * 128 : (t + 1) * 128, :])
                nc.vector.tensor_mul(d[:], d[:], mt[:])
                nc.scalar.dma_start(out[t * 128 : (t + 1) * 128, :], d[:])
    nc.finalize()
    return nc


def _get_program():
    if "nc" not in _cache:
        _cache["nc"] = _build_program()
    return _cache["nc"]


def _compute_mask(cv: float) -> np.ndarray:
    """Replicates reference's mask math in numpy f32: [N, N]."""
    template = np.linspace(1.0 - MAX_SIZE, 0.0, MAX_SIZE, dtype=np.float32)
    one_d = np.clip(
        (template + np.float32(cv) * MAX_SIZE) / np.float32(RAMP_SIZE) + np.float32(1.0),
        np.float32(0.0),
        np.float32(1.0),
    ).astype(np.float32)
    one_d = one_d[-(N // 2):]  # [128]
    idx = np.arange(N)
    ring = np.minimum(
        np.minimum(idx[:, None], idx[None, :]),
        np.minimum(N - 1 - idx[:, None], N - 1 - idx[None, :]),
    )  # values in [0, 127] for N=256 — always < N//2, no center special case
    return one_d[ring]


def _run(x, current_val, **spmd_kwargs):
    x = np.ascontiguousarray(np.asarray(x), dtype=np.float32)
    cv = float(np.asarray(current_val).reshape(-1)[0])
    assert x.shape == (B, H, N, N), x.shape

    mask = _compute_mask(cv)  # [256, 256]
    # unique mask in tile layout: row p%16 of every SBUF partition group
    m4 = np.ascontiguousarray(mask.reshape(16, TILE_F))
    # 0/1 partition-broadcast selector for the PE: W[k, p] = (p % 16 == k)
    wsel = np.ascontiguousarray(
        (np.arange(128)[None, :] % 16 == np.arange(16)[:, None]).astype(np.float32)
    )

    per_core = B // N_CORES
    in_maps = [
        {
            "x_in": x[c * per_core : (c + 1) * per_core].reshape(ROWS, TILE_F),
            "m_in": m4,
            "w_in": wsel,
        }
        for c in range(N_CORES)
    ]

    nc = _get_program()
    res = run_bass_kernel_spmd(nc, in_maps, list(range(N_CORES)), **spmd_kwargs)
    out = np.concatenate(
        [r["out"].reshape(per_core, H, N, N) for r in res.results], axis=0
    )
    return out, res


def kernel(x, current_val):
    return _run(x, current_val)[0]


if __name__ == "__main__":
    xs = np.random.randn(B, H, N, N).astype(np.float32)
    cv = np.array([0.1], dtype=np.float32)
    o = kernel(x=xs, current_val=cv)
    expected = xs * _compute_mask(0.1)
    print("self-check max abs diff:", np.abs(o - expected).max())
